# revision 25
# baseline (speedup 1.0000x reference)
"""DTW kernel for nn_DTW_56272661512310 — Bass/Tile implementation for TRN2.

Sharding: data-parallel over batch B=64 across 8 NeuronCores (8 samples per
core); scalars a, b replicated.

Per-core math (b=8 samples, I=512, J=384, D=768):
  sim = tanh(a * cos_sim(emb1, emb2) + b)            [b, I, J]
  DTW DP (row-scan reformulation with M[i,j] = cummax_i P[:,j]):
      M[i,j] = max(M[i-1,j], M[i-1,j-1] + relu'(sim[i,j]))
      M[i,0] = max(M[i-1,0], sim[i,0])   (raw col 0)
  ans = (M[I-2, J-2] + relu(sim[I-1, J-1])) / J

On-chip layout for the DP: shifted columns J' = j-1 in [0, 383), split into
15 blocks of W=26 per sample. Partition p = 16*s + blk holds block blk of
sample s; p = 16*s + 15 is a "feeder" lane that carries the j=0 raw-column
cummax (G) so the per-step cross-partition shift is a single 32-lane
stream_shuffle with no extra fixup ops.

Engines: PE does the cosine matmul (bf16), ACT does squares/norms/tanh/relu,
DVE runs the 510-step DP (shuffle + add + max per row), DMA rearranges the
sim matrix into DP layout.
"""

import numpy as np
from contextlib import ExitStack

B, I, J, D = 64, 512, 384, 768
N_CORES = 8
BPC = B // N_CORES          # samples per core = 8
W = 26                      # DP block width (shifted-j domain)
NBLK = 15                   # real blocks per sample (15*26 = 390 >= 383)
SPP = 16                    # partitions per sample (15 real + 1 feeder)
NEG = -1.0e30
EPS = 1e-8

_CACHE = {}


def _build_nc():
    import concourse.bass as bass
    import concourse.bacc as bacc
    import concourse.tile as tile
    import concourse.mybir as mybir

    f32 = mybir.dt.float32
    bf16 = mybir.dt.bfloat16
    AF = mybir.ActivationFunctionType

    nc = bacc.Bacc("TRN2", target_bir_lowering=False, debug=False)

    e1_d = nc.dram_tensor("e1", [BPC, I, D], f32, kind="ExternalInput")
    e2_d = nc.dram_tensor("e2", [BPC, J, D], f32, kind="ExternalInput")
    ab_d = nc.dram_tensor("ab", [128, 2], f32, kind="ExternalInput")
    out_d = nc.dram_tensor("out", [BPC, 1], f32, kind="ExternalOutput")

    MI = I // 128            # 4 i-chunks per sample
    MJ = J // 128            # 3 j-chunks per sample
    DC = D // 128            # 6 d-chunks

    # stream_shuffle mask (per 32-lane quadrant, applied to all 4 quadrants):
    # dest d <- d-1 (block shift within sample), dest 0 <- 15 (feeder of
    # sample A), dest 16 <- 31 (feeder of sample B).
    mask = [15 if d == 0 else (31 if d == 16 else d - 1) for d in range(32)]

    with ExitStack() as ctx:
        tc = ctx.enter_context(tile.TileContext(nc))
        const = ctx.enter_context(tc.tile_pool(name="const", bufs=1))
        persist = ctx.enter_context(tc.tile_pool(name="persist", bufs=1))
        stage = ctx.enter_context(tc.tile_pool(name="stage", bufs=3))
        tpool = ctx.enter_context(tc.tile_pool(name="tpool", bufs=2))
        rpool = ctx.enter_context(tc.tile_pool(name="rpool", bufs=2))
        psum = ctx.enter_context(
            tc.tile_pool(name="psum", bufs=8, space="PSUM")
        )

        # ---- scalars a, b (host passes them pre-broadcast [128, 2]) ----
        ab_sb = const.tile([128, 2], f32)
        nc.sync.dma_start(ab_sb[:], ab_d[:])
        a_bc = ab_sb[:, 0:1]
        b_bc = ab_sb[:, 1:2]

        # ---- persistent DP storage --------------------------------------
        # R_dp[p, i, c]: p = 16*s + blk; real cells (blk<15, 26*blk+c+1<=383)
        # hold relu(sim[i, 26*blk+c+1]); feeder lane (blk=15) has c<25 = NEG
        # and c=25 = raw sim[i, 0].
        R_dp = persist.tile([128, I, W], f32)
        S = persist.tile([128, W + 1], f32)
        A = persist.tile([128, W], f32)

        # one-time constant regions (feeder lanes and block-14 pad cells).
        # A [128, W] NEG tile is DMA-broadcast (0-stride source dim) into the
        # per-sample lanes; these have no producers, so Tile schedules them
        # early and they overlap with the input DMAs.
        neg_t = const.tile([128, W], f32)
        nc.vector.memset(neg_t[:], NEG)
        for s in range(BPC):
            nc.gpsimd.dma_start(
                R_dp[SPP * s + 15:SPP * s + 16, :, 0:25],
                neg_t[0:1, 0:25].unsqueeze(1).to_broadcast([1, I, 25]))
            nc.gpsimd.dma_start(
                R_dp[SPP * s + 14:SPP * s + 15, :, 19:26],
                neg_t[0:1, 0:7].unsqueeze(1).to_broadcast([1, I, 7]))

        # ================================================================
        # Phase A: per-sample similarity matrix
        # ================================================================
        for s in range(BPC):
            # -- load e1 natural [128i, m, 768d], e2 natural [128j, q, 768d]
            # (one DMA per tensor keeps per-DMA wait counts low)
            e1n = stage.tile([128, MI, D], f32, name=f"e1n_{s}", tag="e1n")
            nc.sync.dma_start(
                e1n[:], e1_d[s].rearrange("(m p) d -> p m d", p=128))
            e2n = stage.tile([128, MJ, D], f32, name=f"e2n_{s}", tag="e2n",
                             bufs=1)
            nc.sync.dma_start(
                e2n[:], e2_d[s].rearrange("(q p) d -> p q d", p=128))

            # -- row sum-of-squares via ACT Square + accum_out
            ss1 = stage.tile([128, MI], f32, name=f"ss1_{s}", tag="ss1")
            ss2 = stage.tile([128, MJ], f32, name=f"ss2_{s}", tag="ss2")
            for m in range(MI):
                sq = stage.tile([128, D], f32, name=f"sq1_{s}_{m}", tag="sq",
                                bufs=1)
                nc.scalar.activation(sq[:], e1n[:, m, :], AF.Square,
                                     accum_out=ss1[:, m:m + 1])
            for q in range(MJ):
                sq = stage.tile([128, D], f32, name=f"sq2_{s}_{q}", tag="sq",
                                bufs=1)
                nc.scalar.activation(sq[:], e2n[:, q, :], AF.Square,
                                     accum_out=ss2[:, q:q + 1])

            # -- u = 1 / max(sqrt(ss), eps); au1 = a * u1
            u1 = stage.tile([128, MI], f32, name=f"u1_{s}", tag="u1")
            u2 = stage.tile([128, MJ], f32, name=f"u2_{s}", tag="u2")
            nc.scalar.activation(u1[:], ss1[:], AF.Sqrt)
            nc.scalar.activation(u2[:], ss2[:], AF.Sqrt)
            nc.vector.tensor_scalar_max(u1[:], u1[:], EPS)
            nc.vector.tensor_scalar_max(u2[:], u2[:], EPS)
            nc.vector.reciprocal(u1[:], u1[:])
            nc.vector.reciprocal(u2[:], u2[:])
            au1 = stage.tile([128, MI], f32, name=f"au1_{s}", tag="au1")
            nc.vector.tensor_mul(au1[:], u1[:], a_bc[:].to_broadcast([128, MI]))

            # -- cast e1 -> bf16 (norm folded into tanh scale); scale+cast e2
            e1b = stage.tile([128, MI, D], bf16, name=f"e1b_{s}", tag="e1b",
                             bufs=1)
            for m in range(MI):
                nc.scalar.activation(e1b[:, m, :], e1n[:, m, :], AF.Copy)
            e2b = stage.tile([128, MJ, D], bf16, name=f"e2b_{s}", tag="e2b",
                             bufs=1)
            for q in range(MJ):
                nc.scalar.activation(e2b[:, q, :], e2n[:, q, :], AF.Copy,
                                     scale=u2[:, q:q + 1])

            # -- transpose to [d-part, row-free] via xbar DMA (one per
            # tensor: in [128, MI*D] -> out [128, MI*DC, 128]; row r of the
            # logical transpose lands at partition r%128, plane r//128; since
            # D % 128 == 0, plane e = DC*m + dc and partition = d % 128).
            # T tiles are unique per sample so the transposes carry a single
            # wait (HWDGE DMA descriptors allow at most 2).
            T1 = tpool.tile([128, MI * DC, 128], bf16, name=f"T1_{s}",
                            tag=f"T1_{s}", bufs=1)
            T2 = tpool.tile([128, MJ * DC, 128], bf16, name=f"T2_{s}",
                            tag=f"T2_{s}", bufs=1)
            nc.sync.dma_start_transpose(
                T1[:], e1b[:].rearrange("p m d -> p (m d)"))
            nc.sync.dma_start_transpose(
                T2[:], e2b[:].rearrange("p m d -> p (m d)"))

            # -- matmul + tanh + relu per i-chunk
            for m in range(MI):
                ps = psum.tile([128, J], f32, name=f"ps_{s}_{m}", tag="ps")
                for dc in range(DC):
                    nc.tensor.matmul(
                        ps[:],
                        T1[:, DC * m + dc, :],
                        T2[:, dc:MJ * DC:DC, :],
                        start=(dc == 0),
                        stop=(dc == DC - 1),
                    )
                rraw = rpool.tile([128, J], f32, name=f"rraw_{s}_{m}",
                                  tag="rraw")
                nc.scalar.activation(rraw[:], ps[:], AF.Tanh,
                                     bias=b_bc[:], scale=au1[:, m:m + 1])
                rrel = rpool.tile([128, J], f32, name=f"rrel_{s}_{m}",
                                  tag="rrel")
                nc.scalar.activation(rrel[:], rraw[:], AF.Relu)

                # -- rearrange into DP layout ---------------------------
                # per block: src [128 i, cols] -> dst [1 blk-lane, 128 i,
                # cols]; C-orders match (i-major both sides).
                for blk in range(NBLK):
                    j0 = 1 + W * blk
                    cols = min(W, J - j0)      # 26, except 19 for blk 14
                    p = SPP * s + blk
                    nc.scalar.dma_start(
                        R_dp[p:p + 1, 128 * m:128 * (m + 1), 0:cols],
                        rrel[:, j0:j0 + cols])
                # feeder G column: raw sim[:, 0]
                nc.scalar.dma_start(
                    R_dp[SPP * s + 15:SPP * s + 16,
                         128 * m:128 * (m + 1), 25:26],
                    rraw[:, 0:1])

        # ================================================================
        # Phase B: the DP scan over rows i = 1..I-2
        # ================================================================
        # init S from row 0; feeder lanes: M[0..24] = 0, M[25] = raw[0,0]
        nc.vector.tensor_copy(S[:, 1:W + 1], R_dp[:, 0, :])
        zero_t = const.tile([1, W], f32)
        nc.vector.memset(zero_t[:], 0.0)
        for s in range(BPC):
            nc.gpsimd.dma_start(S[SPP * s + 15:SPP * s + 16, 1:W],
                                zero_t[0:1, 0:W - 1])

        for i in range(1, I - 1):
            nc.vector.stream_shuffle(S[:, 0:1], S[:, W:W + 1], mask)
            nc.vector.tensor_add(A[:], S[:, 0:W], R_dp[:, i, :])
            nc.vector.tensor_max(S[:, 1:W + 1], S[:, 1:W + 1], A[:])

        # ---- answer: (M[I-2, 381'] + relu(sim[I-1, 383])) / J ----------
        # J' = 381 = 26*14 + 17 -> lane 16s+14, S col 18.
        g = const.tile([BPC, 2], f32)
        for s in range(BPC):
            p = SPP * s + 14
            nc.gpsimd.dma_start(g[s:s + 1, 0:1], S[p:p + 1, 18:19])
            nc.gpsimd.dma_start(g[s:s + 1, 1:2], R_dp[p:p + 1, I - 1, 18:19])
        ans = const.tile([BPC, 1], f32)
        nc.vector.tensor_add(ans[:], g[:, 0:1], g[:, 1:2])
        nc.vector.tensor_scalar_mul(ans[:], ans[:], 1.0 / J)
        nc.gpsimd.dma_start(out_d[:], ans[:])

    nc.compile()
    return nc


def _get_nc():
    if "nc" not in _CACHE:
        _CACHE["nc"] = _build_nc()
    return _CACHE["nc"]


def _in_maps(emb1, emb2, a, b):
    e1 = np.ascontiguousarray(
        np.asarray(emb1, dtype=np.float32).reshape(N_CORES, BPC, I, D))
    e2 = np.ascontiguousarray(
        np.asarray(emb2, dtype=np.float32).reshape(N_CORES, BPC, J, D))
    ab = np.broadcast_to(
        np.array([np.float32(np.asarray(a).reshape(-1)[0]),
                  np.float32(np.asarray(b).reshape(-1)[0])],
                 dtype=np.float32), (128, 2)).copy()
    return [
        {"e1": e1[c], "e2": e2[c], "ab": ab}
        for c in range(N_CORES)
    ]


def kernel(emb1, emb2, a, b):
    from concourse import bass_utils

    nc = _get_nc()
    res = bass_utils.run_bass_kernel_spmd(
        nc, _in_maps(emb1, emb2, a, b), core_ids=list(range(N_CORES)))
    out = np.concatenate(
        [res.results[c]["out"].reshape(BPC) for c in range(N_CORES)])
    return out.astype(np.float32)


# ---------------------------------------------------------------------------
# numpy reference of the same per-core math, for CoreSim validation
# ---------------------------------------------------------------------------
def _ref_core(e1, e2, a, b):
    n1 = e1 / np.maximum(np.linalg.norm(e1, axis=-1, keepdims=True), EPS)
    n2 = e2 / np.maximum(np.linalg.norm(e2, axis=-1, keepdims=True), EPS)
    sim = np.tanh(np.einsum("bid,bjd->bij", n1, n2) * a + b)
    R = np.maximum(sim, 0.0)
    Rp = R.copy()
    Rp[:, :, 0] = sim[:, :, 0]
    M = Rp[:, 0, :].copy()
    H = np.empty_like(M)
    for i in range(1, I - 1):
        H[:, 1:] = M[:, :-1]
        H[:, 0] = 0.0
        np.maximum(M, H + Rp[:, i, :], out=M)
    return (M[:, J - 2] + R[:, I - 1, J - 1]) / np.float32(J)


def _sim_check(seed=0, tol=2e-2):
    from concourse.bass_interp import CoreSim

    rng = np.random.default_rng(seed)
    e1 = rng.standard_normal((BPC, I, D), dtype=np.float32)
    e2 = rng.standard_normal((BPC, J, D), dtype=np.float32)
    a = np.float32(rng.random())
    b = np.float32(rng.random())

    nc = _get_nc()
    sim = CoreSim(nc)
    sim.tensor("e1")[:] = e1
    sim.tensor("e2")[:] = e2
    sim.tensor("ab")[:] = np.broadcast_to(np.array([a, b], dtype=np.float32), (128, 2))
    sim.simulate(check_with_hw=False)
    got = np.asarray(sim.tensor("out")).reshape(BPC)
    want = _ref_core(e1, e2, a, b)
    err = np.abs(got - want).max() / max(np.abs(want).max(), 1e-9)
    print("sim out :", got)
    print("ref out :", want)
    print("rel err :", err)
    assert err < tol, f"sim mismatch: {err}"
    return err


if __name__ == "__main__":
    _sim_check()


# revision 27
# speedup vs baseline: 54.9589x; 54.9589x over previous
"""DTW kernel for nn_DTW_56272661512310 — Bass/Tile implementation for TRN2.

Sharding: data-parallel over batch B=64 across 8 NeuronCores (8 samples per
core); scalars a, b replicated.

Per-core math (b=8 samples, I=512, J=384, D=768):
  sim = tanh(a * cos_sim(emb1, emb2) + b)            [b, I, J]
  DTW DP (row-scan reformulation with M[i,j] = cummax_i P[:,j]):
      M[i,j] = max(M[i-1,j], M[i-1,j-1] + relu'(sim[i,j]))
      M[i,0] = max(M[i-1,0], sim[i,0])   (raw col 0)
  ans = (M[I-2, J-2] + relu(sim[I-1, J-1])) / J

On-chip layout for the DP: shifted columns J' = j-1 in [0, 383), split into
15 blocks of W=26 per sample. Partition p = 16*s + blk holds block blk of
sample s; p = 16*s + 15 is a "feeder" lane that carries the j=0 raw-column
cummax (G) so the per-step cross-partition shift is a single 32-lane
stream_shuffle with no extra fixup ops.

Engines: PE does the cosine matmul (bf16), ACT does squares/norms/tanh/relu,
DVE runs the 510-step DP (shuffle + add + max per row), DMA rearranges the
sim matrix into DP layout.
"""

import numpy as np
from contextlib import ExitStack

B, I, J, D = 64, 512, 384, 768
N_CORES = 8
BPC = B // N_CORES          # samples per core = 8
W = 26                      # DP block width (shifted-j domain)
NBLK = 15                   # real blocks per sample (15*26 = 390 >= 383)
SPP = 16                    # partitions per sample (15 real + 1 feeder)
NEG = -1.0e30
EPS = 1e-8

_CACHE = {}


def _build_nc():
    import concourse.bass as bass
    import concourse.bacc as bacc
    import concourse.tile as tile
    import concourse.mybir as mybir

    f32 = mybir.dt.float32
    bf16 = mybir.dt.bfloat16
    AF = mybir.ActivationFunctionType

    nc = bacc.Bacc("TRN2", target_bir_lowering=False, debug=False)

    e1_d = nc.dram_tensor("e1", [BPC, I, D], f32, kind="ExternalInput")
    e2_d = nc.dram_tensor("e2", [BPC, J, D], f32, kind="ExternalInput")
    ab_d = nc.dram_tensor("ab", [128, 2], f32, kind="ExternalInput")
    out_d = nc.dram_tensor("out", [BPC, 1], f32, kind="ExternalOutput")

    MI = I // 128            # 4 i-chunks per sample
    MJ = J // 128            # 3 j-chunks per sample
    DC = D // 128            # 6 d-chunks

    # stream_shuffle mask (per 32-lane quadrant, applied to all 4 quadrants):
    # dest d <- d-1 (block shift within sample), dest 0 <- 15 (feeder of
    # sample A), dest 16 <- 31 (feeder of sample B).
    mask = [15 if d == 0 else (31 if d == 16 else d - 1) for d in range(32)]

    with ExitStack() as ctx:
        tc = ctx.enter_context(tile.TileContext(nc))
        const = ctx.enter_context(tc.tile_pool(name="const", bufs=1))
        persist = ctx.enter_context(tc.tile_pool(name="persist", bufs=1))
        stage = ctx.enter_context(tc.tile_pool(name="stage", bufs=3))
        tpool = ctx.enter_context(tc.tile_pool(name="tpool", bufs=2))
        rpool = ctx.enter_context(tc.tile_pool(name="rpool", bufs=2))
        psum = ctx.enter_context(
            tc.tile_pool(name="psum", bufs=8, space="PSUM")
        )

        # ---- scalars a, b (host passes them pre-broadcast [128, 2]) ----
        ab_sb = const.tile([128, 2], f32)
        nc.sync.dma_start(ab_sb[:], ab_d[:])
        a_bc = ab_sb[:, 0:1]
        b_bc = ab_sb[:, 1:2]

        # ---- persistent DP storage --------------------------------------
        # R_dp[p, i, c]: p = 16*s + blk; real cells (blk<15, 26*blk+c+1<=383)
        # hold relu(sim[i, 26*blk+c+1]); feeder lane (blk=15) has c<25 = NEG
        # and c=25 = raw sim[i, 0].
        R_dp = persist.tile([128, I, W], f32)
        S = persist.tile([128, W + 1], f32)
        A = persist.tile([128, W], f32)

        # one-time constant regions (feeder lanes and block-14 pad cells).
        # A [128, W] NEG tile is DMA-broadcast (0-stride source dim) into the
        # per-sample lanes; these have no producers, so Tile schedules them
        # early and they overlap with the input DMAs.
        neg_t = const.tile([128, W], f32)
        nc.vector.memset(neg_t[:], NEG)
        for s in range(BPC):
            nc.gpsimd.dma_start(
                R_dp[SPP * s + 15:SPP * s + 16, :, 0:25],
                neg_t[0:1, 0:25].unsqueeze(1).to_broadcast([1, I, 25]))
            nc.gpsimd.dma_start(
                R_dp[SPP * s + 14:SPP * s + 15, :, 19:26],
                neg_t[0:1, 0:7].unsqueeze(1).to_broadcast([1, I, 7]))

        # ================================================================
        # Phase A: per-sample similarity matrix
        # ================================================================
        for s in range(BPC):
            # -- load e1 natural [128i, m, 768d], e2 natural [128j, q, 768d]
            # (one DMA per tensor keeps per-DMA wait counts low)
            e1n = stage.tile([128, MI, D], f32, name=f"e1n_{s}", tag="e1n")
            nc.sync.dma_start(
                e1n[:], e1_d[s].rearrange("(m p) d -> p m d", p=128))
            e2n = stage.tile([128, MJ, D], f32, name=f"e2n_{s}", tag="e2n",
                             bufs=1)
            nc.sync.dma_start(
                e2n[:], e2_d[s].rearrange("(q p) d -> p q d", p=128))

            # -- row sum-of-squares via ACT Square + accum_out
            ss1 = stage.tile([128, MI], f32, name=f"ss1_{s}", tag="ss1")
            ss2 = stage.tile([128, MJ], f32, name=f"ss2_{s}", tag="ss2")
            for m in range(MI):
                sq = stage.tile([128, D], f32, name=f"sq1_{s}_{m}", tag="sq",
                                bufs=1)
                nc.scalar.activation(sq[:], e1n[:, m, :], AF.Square,
                                     accum_out=ss1[:, m:m + 1])
            for q in range(MJ):
                sq = stage.tile([128, D], f32, name=f"sq2_{s}_{q}", tag="sq",
                                bufs=1)
                nc.scalar.activation(sq[:], e2n[:, q, :], AF.Square,
                                     accum_out=ss2[:, q:q + 1])

            # -- u = 1 / max(sqrt(ss), eps); au1 = a * u1
            u1 = stage.tile([128, MI], f32, name=f"u1_{s}", tag="u1")
            u2 = stage.tile([128, MJ], f32, name=f"u2_{s}", tag="u2")
            nc.scalar.activation(u1[:], ss1[:], AF.Sqrt)
            nc.scalar.activation(u2[:], ss2[:], AF.Sqrt)
            nc.vector.tensor_scalar_max(u1[:], u1[:], EPS)
            nc.vector.tensor_scalar_max(u2[:], u2[:], EPS)
            nc.vector.reciprocal(u1[:], u1[:])
            nc.vector.reciprocal(u2[:], u2[:])
            au1 = stage.tile([128, MI], f32, name=f"au1_{s}", tag="au1")
            nc.vector.tensor_mul(au1[:], u1[:], a_bc[:].to_broadcast([128, MI]))

            # -- cast e1 -> bf16 (norm folded into tanh scale); scale+cast e2
            e1b = stage.tile([128, MI, D], bf16, name=f"e1b_{s}", tag="e1b",
                             bufs=1)
            for m in range(MI):
                nc.scalar.activation(e1b[:, m, :], e1n[:, m, :], AF.Copy)
            e2b = stage.tile([128, MJ, D], bf16, name=f"e2b_{s}", tag="e2b",
                             bufs=1)
            for q in range(MJ):
                nc.scalar.activation(e2b[:, q, :], e2n[:, q, :], AF.Copy,
                                     scale=u2[:, q:q + 1])

            # -- transpose to [d-part, row-free] via xbar DMA (one per
            # tensor: in [128, MI*D] -> out [128, MI*DC, 128]; row r of the
            # logical transpose lands at partition r%128, plane r//128; since
            # D % 128 == 0, plane e = DC*m + dc and partition = d % 128).
            # T tiles are unique per sample so the transposes carry a single
            # wait (HWDGE DMA descriptors allow at most 2).
            T1 = tpool.tile([128, MI * DC, 128], bf16, name=f"T1_{s}",
                            tag=f"T1_{s}", bufs=1)
            T2 = tpool.tile([128, MJ * DC, 128], bf16, name=f"T2_{s}",
                            tag=f"T2_{s}", bufs=1)
            nc.sync.dma_start_transpose(
                T1[:], e1b[:].rearrange("p m d -> p (m d)"))
            nc.sync.dma_start_transpose(
                T2[:], e2b[:].rearrange("p m d -> p (m d)"))

            # -- matmul + tanh + relu per i-chunk
            for m in range(MI):
                ps = psum.tile([128, J], f32, name=f"ps_{s}_{m}", tag="ps")
                for dc in range(DC):
                    nc.tensor.matmul(
                        ps[:],
                        T1[:, DC * m + dc, :],
                        T2[:, dc:MJ * DC:DC, :],
                        start=(dc == 0),
                        stop=(dc == DC - 1),
                    )
                rraw = rpool.tile([128, J], f32, name=f"rraw_{s}_{m}",
                                  tag="rraw")
                nc.scalar.activation(rraw[:], ps[:], AF.Tanh,
                                     bias=b_bc[:], scale=au1[:, m:m + 1])
                rrel = rpool.tile([128, J], f32, name=f"rrel_{s}_{m}",
                                  tag="rrel")
                nc.scalar.activation(rrel[:], rraw[:], AF.Relu)

                # -- rearrange into DP layout ---------------------------
                # per block: src [128 i, cols] -> dst [1 blk-lane, 128 i,
                # cols]; C-orders match (i-major both sides).
                for blk in range(NBLK):
                    j0 = 1 + W * blk
                    cols = min(W, J - j0)      # 26, except 19 for blk 14
                    p = SPP * s + blk
                    nc.scalar.dma_start(
                        R_dp[p:p + 1, 128 * m:128 * (m + 1), 0:cols],
                        rrel[:, j0:j0 + cols])
                # feeder G column: raw sim[:, 0]
                nc.scalar.dma_start(
                    R_dp[SPP * s + 15:SPP * s + 16,
                         128 * m:128 * (m + 1), 25:26],
                    rraw[:, 0:1])

        # ================================================================
        # Phase B: the DP scan over rows i = 1..I-2
        # ================================================================
        # init S from row 0; feeder lanes: M[0..24] = 0, M[25] = raw[0,0]
        nc.vector.tensor_copy(S[:, 1:W + 1], R_dp[:, 0, :])
        zero_t = const.tile([1, W], f32)
        nc.vector.memset(zero_t[:], 0.0)
        for s in range(BPC):
            nc.gpsimd.dma_start(S[SPP * s + 15:SPP * s + 16, 1:W],
                                zero_t[0:1, 0:W - 1])

        for i in range(1, I - 1):
            nc.vector.stream_shuffle(S[:, 0:1], S[:, W:W + 1], mask)
            nc.vector.tensor_add(A[:], S[:, 0:W], R_dp[:, i, :])
            nc.vector.tensor_max(S[:, 1:W + 1], S[:, 1:W + 1], A[:])

        # ---- answer: (M[I-2, 381'] + relu(sim[I-1, 383])) / J ----------
        # J' = 381 = 26*14 + 17 -> lane 16s+14, S col 18.
        g = const.tile([BPC, 2], f32)
        for s in range(BPC):
            p = SPP * s + 14
            nc.gpsimd.dma_start(g[s:s + 1, 0:1], S[p:p + 1, 18:19])
            nc.gpsimd.dma_start(g[s:s + 1, 1:2], R_dp[p:p + 1, I - 1, 18:19])
        ans = const.tile([BPC, 1], f32)
        nc.vector.tensor_add(ans[:], g[:, 0:1], g[:, 1:2])
        nc.vector.tensor_scalar_mul(ans[:], ans[:], 1.0 / J)
        nc.gpsimd.dma_start(out_d[:], ans[:])

    nc.compile()
    return nc


def _get_nc():
    if "nc" not in _CACHE:
        _CACHE["nc"] = _build_nc()
    return _CACHE["nc"]


def _in_maps(emb1, emb2, a, b):
    e1 = np.ascontiguousarray(
        np.asarray(emb1, dtype=np.float32).reshape(N_CORES, BPC, I, D))
    e2 = np.ascontiguousarray(
        np.asarray(emb2, dtype=np.float32).reshape(N_CORES, BPC, J, D))
    ab = np.broadcast_to(
        np.array([np.float32(np.asarray(a).reshape(-1)[0]),
                  np.float32(np.asarray(b).reshape(-1)[0])],
                 dtype=np.float32), (128, 2)).copy()
    return [
        {"e1": e1[c], "e2": e2[c], "ab": ab}
        for c in range(N_CORES)
    ]


def _get_exec():
    """Build (once) a jitted shard_map executor over the 8 cores.

    Mirrors bass2jax.run_bass_via_pjrt but caches the jitted callable so
    repeat kernel() calls skip retrace/recompile, and exposes the pieces
    needed to time execution with device-resident inputs.
    """
    if "exec" in _CACHE:
        return _CACHE["exec"]

    import jax
    from jax.experimental.shard_map import shard_map
    from jax.sharding import Mesh, PartitionSpec
    import concourse.mybir as mybir
    from concourse.bass2jax import (
        _bass_exec_p, install_neuronx_cc_hook, partition_id_tensor)

    nc = _get_nc()
    install_neuronx_cc_hook()

    partition_name = (nc.partition_id_tensor.name
                      if nc.partition_id_tensor else None)
    in_names, out_names, out_avals = [], [], []
    for alloc in nc.m.functions[0].allocations:
        if not isinstance(alloc, mybir.MemoryLocationSet):
            continue
        if not alloc.memorylocations:
            continue
        name = alloc.memorylocations[0].name
        if alloc.kind == "ExternalInput":
            if name != partition_name:
                in_names.append(name)
        elif alloc.kind == "ExternalOutput":
            out_names.append(name)
            out_avals.append(jax.core.ShapedArray(
                tuple(alloc.tensor_shape), mybir.dt.np(alloc.dtype)))
    n_params = len(in_names)
    all_names = in_names + out_names
    if partition_name is not None:
        all_names = all_names + [partition_name]

    def _body(*args):
        operands = list(args)
        if partition_name is not None:
            operands.append(partition_id_tensor())
        outs = _bass_exec_p.bind(
            *operands,
            out_avals=tuple(out_avals),
            in_names=tuple(all_names),
            out_names=tuple(out_names),
            lowering_input_output_aliases=(),
            sim_require_finite=True,
            sim_require_nnan=True,
            nc=nc,
        )
        return tuple(outs)

    devices = jax.devices()[:N_CORES]
    mesh = Mesh(np.asarray(devices), ("core",))
    n_outs = len(out_names)
    sharded = jax.jit(
        shard_map(
            _body, mesh=mesh,
            in_specs=(PartitionSpec("core"),) * (n_params + n_outs),
            out_specs=(PartitionSpec("core"),) * n_outs,
            check_rep=False),
        donate_argnums=tuple(range(n_params, n_params + n_outs)),
        keep_unused=True)

    ex = {
        "fn": sharded, "mesh": mesh, "in_names": in_names,
        "out_names": out_names, "out_avals": out_avals,
    }
    _CACHE["exec"] = ex
    return ex


def _concat_inputs(in_maps, ex):
    return [
        np.concatenate([np.asarray(in_maps[c][name]) for c in range(N_CORES)],
                       axis=0)
        for name in ex["in_names"]
    ]


def _zero_outs(ex):
    return [
        np.zeros((N_CORES * av.shape[0], *av.shape[1:]), av.dtype)
        for av in ex["out_avals"]
    ]


def kernel(emb1, emb2, a, b):
    ex = _get_exec()
    concat_in = _concat_inputs(_in_maps(emb1, emb2, a, b), ex)
    out_arrs = ex["fn"](*concat_in, *_zero_outs(ex))
    out = np.asarray(out_arrs[ex["out_names"].index("out")])
    return out.reshape(B).astype(np.float32)


# ---------------------------------------------------------------------------
# numpy reference of the same per-core math, for CoreSim validation
# ---------------------------------------------------------------------------
def _ref_core(e1, e2, a, b):
    n1 = e1 / np.maximum(np.linalg.norm(e1, axis=-1, keepdims=True), EPS)
    n2 = e2 / np.maximum(np.linalg.norm(e2, axis=-1, keepdims=True), EPS)
    sim = np.tanh(np.einsum("bid,bjd->bij", n1, n2) * a + b)
    R = np.maximum(sim, 0.0)
    Rp = R.copy()
    Rp[:, :, 0] = sim[:, :, 0]
    M = Rp[:, 0, :].copy()
    H = np.empty_like(M)
    for i in range(1, I - 1):
        H[:, 1:] = M[:, :-1]
        H[:, 0] = 0.0
        np.maximum(M, H + Rp[:, i, :], out=M)
    return (M[:, J - 2] + R[:, I - 1, J - 1]) / np.float32(J)


def _sim_check(seed=0, tol=2e-2):
    from concourse.bass_interp import CoreSim

    rng = np.random.default_rng(seed)
    e1 = rng.standard_normal((BPC, I, D), dtype=np.float32)
    e2 = rng.standard_normal((BPC, J, D), dtype=np.float32)
    a = np.float32(rng.random())
    b = np.float32(rng.random())

    nc = _get_nc()
    sim = CoreSim(nc)
    sim.tensor("e1")[:] = e1
    sim.tensor("e2")[:] = e2
    sim.tensor("ab")[:] = np.broadcast_to(np.array([a, b], dtype=np.float32), (128, 2))
    sim.simulate(check_with_hw=False)
    got = np.asarray(sim.tensor("out")).reshape(BPC)
    want = _ref_core(e1, e2, a, b)
    err = np.abs(got - want).max() / max(np.abs(want).max(), 1e-9)
    print("sim out :", got)
    print("ref out :", want)
    print("rel err :", err)
    assert err < tol, f"sim mismatch: {err}"
    return err


if __name__ == "__main__":
    _sim_check()
